# revision 4
# baseline (speedup 1.0000x reference)
"""Trainium2 Bass kernel for 16-head causal MHA (RMSNorm+RoPE on q,k).

Tensor-parallel over heads: 8 cores x 2 heads each. Each core computes
qkv projection for its heads, norm+rope, causal attention, and a partial
out-projection; the host sums the 8 partial outputs.

Layout notes:
- L = 2048 seq, D = 2048 hidden, 16 heads x 128 head_dim.
- Scores are computed transposed [k, q] so softmax denominator comes from a
  ones-vector matmul and PV needs no transposes of the attention matrix.
- RoPE is computed into a de-interleaved [odd-half | even-half] d-order,
  identically for q and k (dot products unchanged); v / out_proj keep the
  natural d-order.
- exp() is computed without max-subtraction: post-RMSNorm |q.k|/sqrt(hd)
  <= sqrt(128), so exp is bounded by ~8.2e4. Masked (upper-triangular)
  score blocks are skipped entirely; diagonal blocks get an additive -1e9.
"""
import os
import numpy as np

import concourse.bacc as bacc
import concourse.mybir as mybir
import concourse.tile as tile
from concourse.bass_utils import run_bass_kernel_spmd

F32 = mybir.dt.float32
F32R = mybir.dt.float32r
AF = mybir.ActivationFunctionType
ALU = mybir.AluOpType
AX = mybir.AxisListType

N_CORES = 8
L = 2048
D = 2048
HD = 128
N_HEAD = 16
HPC = N_HEAD // N_CORES  # heads per core = 2
LT = 128                 # L-tile rows
NT = L // LT             # 16 L-tiles
HC = 128                 # hid chunk
NHC = D // HC            # 16 hid chunks
QT = 512                 # q-tile width in attention
NQT = L // QT            # 4
EPS = 1e-5
ROPE_BASE = 10000.0
SCALE = 1.0 / float(np.sqrt(HD))
NEG = -1.0e9


def build():
    nc = bacc.Bacc("TRN2", target_bir_lowering=False, debug=False,
                   enable_asserts=False, num_devices=N_CORES)

    # Per-core external inputs (host-prepped layouts; see prep_inputs()).
    xt = nc.dram_tensor("xt", [NT, NHC, HC, LT], F32R, kind="ExternalInput")
    wt = nc.dram_tensor("wt", [D, 6 * HD], F32R, kind="ExternalInput")
    wo = nc.dram_tensor("wo", [HPC, HD, D], F32R, kind="ExternalInput")
    w1 = nc.dram_tensor("w1", [NT, LT, HD], F32, kind="ExternalInput")
    w2 = nc.dram_tensor("w2", [NT, LT, HD], F32, kind="ExternalInput")
    mask4 = nc.dram_tensor("mask4", [128, 4, QT], F32, kind="ExternalInput")
    ident_in = nc.dram_tensor("ident", [128, 128], F32R, kind="ExternalInput")
    ones_in = nc.dram_tensor("ones", [128, 1], F32R, kind="ExternalInput")

    out = nc.dram_tensor("out", [L, D], F32, kind="ExternalOutput")

    with tile.TileContext(nc) as tc:
        with (
            tc.tile_pool(name="const", bufs=1) as constp,
            tc.tile_pool(name="wpool", bufs=1) as wpool,
            tc.tile_pool(name="persist", bufs=1) as persist,
            tc.tile_pool(name="dram", bufs=2, space="DRAM") as dramp,
        ):
            # ---- constants ----
            ident = constp.tile([128, 128], F32R)
            nc.sync.dma_start(out=ident, in_=ident_in[:, :])
            ones = constp.tile([128, 1], F32R)
            nc.sync.dma_start(out=ones, in_=ones_in[:, :])
            eps_sb = constp.tile([128, 1], F32)
            nc.vector.memset(eps_sb, EPS)
            mask_sb = constp.tile([128, 4, QT], F32)
            nc.sync.dma_start(out=mask_sb, in_=mask4[:, :, :])
            w1_sb = constp.tile([128, NT, HD], F32)
            nc.sync.dma_start(out=w1_sb, in_=w1.rearrange("t p d -> p t d"))
            w2_sb = constp.tile([128, NT, HD], F32)
            nc.sync.dma_start(out=w2_sb, in_=w2.rearrange("t p d -> p t d"))

            # ---- weights resident ----
            w_sb = wpool.tile([128, NHC, 6 * HD], F32R)
            nc.sync.dma_start(out=w_sb, in_=wt.rearrange("(c p) e -> p c e", p=128))
            wo_sb = wpool.tile([128, HPC, D], F32R)
            nc.sync.dma_start(out=wo_sb, in_=wo.rearrange("h p e -> p h e"))

            # persistent activations
            v_sb = persist.tile([128, NT, HPC * HD], F32R)       # [L-part, t, v cols]
            qT = persist.tile([128, HPC, L], F32R)               # [d, h, L]
            kT = persist.tile([128, HPC, L], F32R)
            outT = persist.tile([128, HPC, L], F32R)             # [d, h, L]
            den_row = persist.tile([1, HPC, L], F32)
            inv_den = persist.tile([128, HPC, NT], F32)

            # ================= Phase 1: QKV + norm + rope + transpose ======
            with (
                tc.tile_pool(name="xin", bufs=2) as xin,
                tc.tile_pool(name="qkv", bufs=3) as qkvp,
                tc.tile_pool(name="ps_qkv", bufs=2, space="PSUM") as ps_qkv,
                tc.tile_pool(name="ps_tr", bufs=2, space="PSUM") as ps_tr,
            ):
                for t in range(NT):
                    x_tile = xin.tile([128, NHC, LT], F32R, tag="x")
                    nc.sync.dma_start(out=x_tile, in_=xt[t].rearrange("c p l -> p c l"))

                    p_qk = ps_qkv.tile([128, 4 * HD], F32, tag="pqk")
                    p_v = ps_qkv.tile([128, HPC * HD], F32, tag="pv")
                    for c in range(NHC):
                        nc.tensor.matmul(p_qk, x_tile[:, c, :], w_sb[:, c, 0:4 * HD],
                                         start=(c == 0), stop=(c == NHC - 1))
                        nc.tensor.matmul(p_v, x_tile[:, c, :], w_sb[:, c, 4 * HD:6 * HD],
                                         start=(c == 0), stop=(c == NHC - 1))

                    # v -> persistent sbuf (natural layout)
                    nc.vector.tensor_copy(v_sb[:, t, :], p_v)

                    # rms-norm scale: s = 1/sqrt(mean(x^2) + eps) per (L, seg)
                    sq = qkvp.tile([128, 4 * HD], F32, tag="sq")
                    nc.scalar.activation(sq, p_qk, AF.Square)
                    ssum = qkvp.tile([128, 4], F32, tag="ssum")
                    nc.vector.reduce_sum(ssum, sq.rearrange("p (g d) -> p g d", g=4),
                                         axis=AX.X)
                    nc.scalar.activation(ssum, ssum, AF.Sqrt, scale=1.0 / HD, bias=eps_sb)
                    s_val = qkvp.tile([128, 4], F32, tag="sval")
                    nc.vector.reciprocal(s_val, ssum)

                    # rope: z = (qk * s) .* W; pairwise-add -> halves
                    roped = qkvp.tile([128, 4 * HD], F32R, tag="roped")
                    for seg in range(4):
                        for half, wtab in ((0, w1_sb), (1, w2_sb)):
                            z = qkvp.tile([128, HD], F32, tag="z")
                            nc.vector.scalar_tensor_tensor(
                                out=z, in0=p_qk[:, seg * HD:(seg + 1) * HD],
                                scalar=s_val[:, seg:seg + 1], in1=wtab[:, t, :],
                                op0=ALU.mult, op1=ALU.mult)
                            with nc.allow_low_precision("2-elem rope pairs"):
                                nc.vector.reduce_sum(
                                    roped[:, seg * HD + half * 64: seg * HD + half * 64 + 64],
                                    z.rearrange("p (d two) -> p d two", two=2),
                                    axis=AX.X)

                    # transpose the 4 roped [128,128] blocks into qT/kT
                    for seg in range(4):
                        tgt = qT if seg < 2 else kT
                        h = seg % 2
                        p_tr = ps_tr.tile([128, 128], F32R, tag="ptr")
                        nc.tensor.transpose(
                            p_tr, roped[:, seg * HD:(seg + 1) * HD], ident)
                        nc.vector.tensor_copy(tgt[:, h, t * LT:(t + 1) * LT], p_tr)

            # ================= Phase 2: attention ==========================
            with (
                tc.tile_pool(name="attn", bufs=3) as attnp,
                tc.tile_pool(name="ps_s", bufs=2, space="PSUM") as ps_s,
                tc.tile_pool(name="ps_o", bufs=2, space="PSUM") as ps_o,
                tc.tile_pool(name="ps_d", bufs=2, space="PSUM") as ps_d,
            ):
                for h in range(HPC):
                    for qt in range(NQT):
                        nkc = 4 * qt + 4
                        p_o = ps_o.tile([128, QT], F32, tag="po")
                        p_den = ps_d.tile([1, QT], F32, tag="pd")
                        for kc in range(nkc):
                            p_s = ps_s.tile([128, QT], F32, tag="ps")
                            nc.tensor.matmul(
                                p_s, kT[:, h, kc * 128:(kc + 1) * 128],
                                qT[:, h, qt * QT:(qt + 1) * QT],
                                start=True, stop=True)
                            if kc >= 4 * qt:
                                nc.vector.tensor_add(
                                    p_s, p_s, mask_sb[:, kc - 4 * qt, :])
                            expT = attnp.tile([128, QT], F32R, tag="expT")
                            nc.scalar.activation(expT, p_s, AF.Exp, scale=SCALE)
                            nc.tensor.matmul(p_den, ones, expT,
                                             start=(kc == 0), stop=(kc == nkc - 1))
                            nc.tensor.matmul(
                                p_o, v_sb[:, kc, h * HD:(h + 1) * HD], expT,
                                start=(kc == 0), stop=(kc == nkc - 1))
                        nc.vector.tensor_copy(outT[:, h, qt * QT:(qt + 1) * QT], p_o)
                        nc.vector.tensor_copy(den_row[:, h, qt * QT:(qt + 1) * QT],
                                              p_den)

                # inv_den: [1, L] per head -> [128, NT] via DRAM bounce
                den_bounce = dramp.tile([1, HPC * L], F32)
                nc.gpsimd.dma_start(out=den_bounce,
                                    in_=den_row.rearrange("o h l -> o (h l)"))
                den_cols = persist.tile([128, HPC, NT], F32)
                nc.gpsimd.dma_start(
                    out=den_cols,
                    in_=den_bounce.rearrange("o (h j p) -> (o p) h j", p=128, j=NT))
                nc.vector.reciprocal(inv_den, den_cols)

            # ================= Phase 3: out projection =====================
            with (
                tc.tile_pool(name="res", bufs=4) as resp,
                tc.tile_pool(name="ps_f", bufs=2, space="PSUM") as ps_f,
            ):
                for t in range(NT):
                    for ec in range(4):
                        p_f0 = ps_f.tile([128, 512], F32, tag="f0")
                        p_f1 = ps_f.tile([128, 512], F32, tag="f1")
                        nc.tensor.matmul(p_f0, outT[:, 0, t * LT:(t + 1) * LT],
                                         wo_sb[:, 0, ec * 512:(ec + 1) * 512],
                                         start=True, stop=True)
                        nc.tensor.matmul(p_f1, outT[:, 1, t * LT:(t + 1) * LT],
                                         wo_sb[:, 1, ec * 512:(ec + 1) * 512],
                                         start=True, stop=True)
                        y = resp.tile([128, 512], F32, tag="y")
                        nc.scalar.activation(y, p_f0, AF.Copy,
                                             scale=inv_den[:, 0, t:t + 1])
                        nc.vector.scalar_tensor_tensor(
                            out=y, in0=p_f1, scalar=inv_den[:, 1, t:t + 1], in1=y,
                            op0=ALU.mult, op1=ALU.add)
                        nc.sync.dma_start(
                            out=out[t * LT:(t + 1) * LT, ec * 512:(ec + 1) * 512],
                            in_=y)
    nc.compile()
    return nc


_NC_CACHE = None


def _get_nc():
    global _NC_CACHE
    if _NC_CACHE is None:
        _NC_CACHE = build()
    return _NC_CACHE


def prep_inputs(x, w_qkv, w_out):
    """Host-side sharding/layout prep. Returns list of per-core input maps."""
    x2d = np.asarray(x, dtype=np.float32).reshape(L, D)
    w_qkv = np.asarray(w_qkv, dtype=np.float32)
    w_out = np.asarray(w_out, dtype=np.float32)

    # xt[t, c, p, l] = x2d[t*128 + l, c*128 + p]
    xt = np.ascontiguousarray(
        x2d.reshape(NT, LT, NHC, HC).transpose(0, 2, 3, 1))

    # rope coefficient tables
    inv_freq = 1.0 / (ROPE_BASE ** (np.arange(0, HD, 2, dtype=np.float64) / HD))
    pos = np.arange(L, dtype=np.float64)[:, None]
    ang = pos * inv_freq[None, :]                       # [L, 64]
    cos, sin = np.cos(ang), np.sin(ang)
    w1 = np.zeros((L, HD), dtype=np.float32)
    w2 = np.zeros((L, HD), dtype=np.float32)
    w1[:, 0::2] = -sin
    w1[:, 1::2] = cos
    w2[:, 0::2] = cos
    w2[:, 1::2] = sin
    w1 = np.ascontiguousarray(w1.reshape(NT, LT, HD))
    w2 = np.ascontiguousarray(w2.reshape(NT, LT, HD))

    # causal mask tiles for diagonal blocks
    i = np.arange(128)[:, None]
    j = np.arange(QT)[None, :]
    mask4 = np.stack(
        [np.where(i + r * 128 <= j, 0.0, NEG).astype(np.float32) for r in range(4)],
        axis=1)  # [128, 4, QT]

    ident = np.eye(128, dtype=np.float32)
    ones = np.ones((128, 1), dtype=np.float32)

    in_maps = []
    for c in range(N_CORES):
        h0 = HPC * c
        rows = []
        for part in range(3):  # q, k, v
            for hh in range(HPC):
                base = part * D + (h0 + hh) * HD
                rows.append(w_qkv[base:base + HD])
        w_c = np.concatenate(rows, axis=0)              # [768, D]
        wt = np.ascontiguousarray(w_c.T)                # [D, 768]
        wo = np.ascontiguousarray(
            w_out[:, h0 * HD:(h0 + HPC) * HD].T.reshape(HPC, HD, D))
        in_maps.append({
            "xt": xt, "wt": wt, "wo": wo, "w1": w1, "w2": w2,
            "mask4": mask4, "ident": ident, "ones": ones,
        })
    return in_maps


def kernel(x, w_qkv, w_out, mask, _trace=False):
    """Full MHA forward. Returns [1, L, D] float32."""
    nc = _get_nc()
    in_maps = prep_inputs(x, w_qkv, w_out)
    res = run_bass_kernel_spmd(nc, in_maps, core_ids=list(range(N_CORES)),
                               trace=_trace)
    acc = np.zeros((L, D), dtype=np.float32)
    for r in res.results:
        acc += r["out"]
    out = acc.reshape(1, L, D)
    if _trace:
        return out, res
    return out


# revision 5
# speedup vs baseline: 1.0464x; 1.0464x over previous
"""Trainium2 Bass kernel for 16-head causal MHA (RMSNorm+RoPE on q,k).

Tensor-parallel over heads: 8 cores x 2 heads each. Each core computes
qkv projection for its heads, norm+rope, causal attention, and a partial
out-projection; the host sums the 8 partial outputs.

Layout notes:
- L = 2048 seq, D = 2048 hidden, 16 heads x 128 head_dim.
- Scores are computed transposed [k, q] so softmax denominator comes from a
  ones-vector matmul and PV needs no transposes of the attention matrix.
- RoPE is computed into a de-interleaved [odd-half | even-half] d-order,
  identically for q and k (dot products unchanged); v / out_proj keep the
  natural d-order.
- exp() is computed without max-subtraction: post-RMSNorm |q.k|/sqrt(hd)
  <= sqrt(128), so exp is bounded by ~8.2e4. Masked (upper-triangular)
  score blocks are skipped entirely; diagonal blocks get an additive -1e9.
"""
import os
import numpy as np

import concourse.bacc as bacc
import concourse.mybir as mybir
import concourse.tile as tile
from concourse.bass_utils import run_bass_kernel_spmd

F32 = mybir.dt.float32
F32R = mybir.dt.float32r
AF = mybir.ActivationFunctionType
ALU = mybir.AluOpType
AX = mybir.AxisListType

N_CORES = 8
L = 2048
D = 2048
HD = 128
N_HEAD = 16
HPC = N_HEAD // N_CORES  # heads per core = 2
LT = 128                 # L-tile rows
NT = L // LT             # 16 L-tiles
HC = 128                 # hid chunk
NHC = D // HC            # 16 hid chunks
QT = 512                 # q-tile width in attention
NQT = L // QT            # 4
EPS = 1e-5
ROPE_BASE = 10000.0
SCALE = 1.0 / float(np.sqrt(HD))
NEG = -1.0e9


def build():
    nc = bacc.Bacc("TRN2", target_bir_lowering=False, debug=False,
                   enable_asserts=False, num_devices=N_CORES)

    # Per-core external inputs (host-prepped layouts; see prep_inputs()).
    xt = nc.dram_tensor("xt", [NT, NHC, HC, LT], F32R, kind="ExternalInput")
    wt = nc.dram_tensor("wt", [D, 6 * HD], F32R, kind="ExternalInput")
    wo = nc.dram_tensor("wo", [HPC, HD, D], F32R, kind="ExternalInput")
    w1 = nc.dram_tensor("w1", [NT, LT, HD], F32, kind="ExternalInput")
    w2 = nc.dram_tensor("w2", [NT, LT, HD], F32, kind="ExternalInput")
    mask4 = nc.dram_tensor("mask4", [128, 4, QT], F32, kind="ExternalInput")
    ident_in = nc.dram_tensor("ident", [128, 128], F32R, kind="ExternalInput")
    ones_in = nc.dram_tensor("ones", [128, 1], F32R, kind="ExternalInput")

    out = nc.dram_tensor("out", [L, D], F32, kind="ExternalOutput")

    with tile.TileContext(nc) as tc:
        with (
            tc.tile_pool(name="const", bufs=1) as constp,
            tc.tile_pool(name="wpool", bufs=1) as wpool,
            tc.tile_pool(name="persist", bufs=1) as persist,
            tc.tile_pool(name="dram", bufs=2, space="DRAM") as dramp,
        ):
            # ---- constants ----
            ident = constp.tile([128, 128], F32R)
            nc.sync.dma_start(out=ident, in_=ident_in[:, :])
            ones = constp.tile([128, 1], F32R)
            nc.sync.dma_start(out=ones, in_=ones_in[:, :])
            eps_sb = constp.tile([128, 1], F32)
            nc.vector.memset(eps_sb, EPS)
            mask_sb = constp.tile([128, 4, QT], F32)
            nc.sync.dma_start(out=mask_sb, in_=mask4[:, :, :])
            w1_sb = constp.tile([128, NT, HD], F32)
            nc.sync.dma_start(out=w1_sb, in_=w1.rearrange("t p d -> p t d"))
            w2_sb = constp.tile([128, NT, HD], F32)
            nc.sync.dma_start(out=w2_sb, in_=w2.rearrange("t p d -> p t d"))

            # ---- weights resident ----
            w_sb = wpool.tile([128, NHC, 6 * HD], F32R)
            nc.sync.dma_start(out=w_sb, in_=wt.rearrange("(c p) e -> p c e", p=128))
            wo_sb = wpool.tile([128, HPC, D], F32R)
            nc.sync.dma_start(out=wo_sb, in_=wo.rearrange("h p e -> p h e"))

            # persistent activations
            v_sb = persist.tile([128, NT, HPC * HD], F32R)       # [L-part, t, v cols]
            qT = persist.tile([128, HPC, L], F32R)               # [d, h, L]
            kT = persist.tile([128, HPC, L], F32R)
            inv_den = persist.tile([128, HPC, NT], F32)

            # ================= Phase 1: QKV + norm + rope + transpose ======
            with (
                tc.tile_pool(name="xin", bufs=3) as xin,
                tc.tile_pool(name="qkv", bufs=3) as qkvp,
                tc.tile_pool(name="ps_qkv", bufs=2, space="PSUM") as ps_qkv,
                tc.tile_pool(name="ps_tr", bufs=2, space="PSUM") as ps_tr,
            ):
                for t in range(NT):
                    x_tile = xin.tile([128, NHC, LT], F32R, tag="x")
                    nc.sync.dma_start(out=x_tile, in_=xt[t].rearrange("c p l -> p c l"))

                    p_qk = ps_qkv.tile([128, 4 * HD], F32, tag="pqk")
                    p_v = ps_qkv.tile([128, HPC * HD], F32, tag="pv")
                    for c in range(NHC):
                        nc.tensor.matmul(p_qk, x_tile[:, c, :], w_sb[:, c, 0:4 * HD],
                                         start=(c == 0), stop=(c == NHC - 1))
                        nc.tensor.matmul(p_v, x_tile[:, c, :], w_sb[:, c, 4 * HD:6 * HD],
                                         start=(c == 0), stop=(c == NHC - 1))

                    # v -> persistent sbuf (natural layout)
                    nc.vector.tensor_copy(v_sb[:, t, :], p_v)

                    # rms-norm scale: s = 1/sqrt(mean(x^2) + eps) per (L, seg)
                    sq = qkvp.tile([128, 4 * HD], F32, tag="sq")
                    nc.scalar.activation(sq, p_qk, AF.Square)
                    ssum = qkvp.tile([128, 4], F32, tag="ssum")
                    nc.vector.reduce_sum(ssum, sq.rearrange("p (g d) -> p g d", g=4),
                                         axis=AX.X)
                    nc.scalar.activation(ssum, ssum, AF.Sqrt, scale=1.0 / HD, bias=eps_sb)
                    s_val = qkvp.tile([128, 4], F32, tag="sval")
                    nc.vector.reciprocal(s_val, ssum)

                    # rope: z = (qk * s) .* W; pairwise-add -> halves
                    roped = qkvp.tile([128, 4 * HD], F32R, tag="roped")
                    for seg in range(4):
                        for half, wtab in ((0, w1_sb), (1, w2_sb)):
                            z = qkvp.tile([128, HD], F32, tag="z")
                            nc.vector.scalar_tensor_tensor(
                                out=z, in0=p_qk[:, seg * HD:(seg + 1) * HD],
                                scalar=s_val[:, seg:seg + 1], in1=wtab[:, t, :],
                                op0=ALU.mult, op1=ALU.mult)
                            with nc.allow_low_precision("2-elem rope pairs"):
                                nc.vector.reduce_sum(
                                    roped[:, seg * HD + half * 64: seg * HD + half * 64 + 64],
                                    z.rearrange("p (d two) -> p d two", two=2),
                                    axis=AX.X)

                    # transpose the 4 roped [128,128] blocks into qT/kT
                    for seg in range(4):
                        tgt = qT if seg < 2 else kT
                        h = seg % 2
                        p_tr = ps_tr.tile([128, 128], F32R, tag="ptr")
                        nc.tensor.transpose(
                            p_tr, roped[:, seg * HD:(seg + 1) * HD], ident)
                        nc.vector.tensor_copy(tgt[:, h, t * LT:(t + 1) * LT], p_tr)

            # ====== Phase 2: attention + out-projection, interleaved per qt ==
            with (
                tc.tile_pool(name="attn", bufs=4) as attnp,
                tc.tile_pool(name="res", bufs=4) as resp,
                tc.tile_pool(name="ps_s", bufs=2, space="PSUM") as ps_s,
                tc.tile_pool(name="ps_o", bufs=2, space="PSUM") as ps_o,
                tc.tile_pool(name="ps_d", bufs=2, space="PSUM") as ps_d,
                tc.tile_pool(name="ps_f", bufs=1, space="PSUM") as ps_f,
            ):
                for qt in range(NQT):
                    o_qt = []
                    for h in range(HPC):
                        nkc = 4 * qt + 4
                        p_o = ps_o.tile([128, QT], F32, tag="po")
                        p_den = ps_d.tile([1, QT], F32, tag="pd")
                        for kc in range(nkc):
                            p_s = ps_s.tile([128, QT], F32, tag="ps")
                            nc.tensor.matmul(
                                p_s, kT[:, h, kc * 128:(kc + 1) * 128],
                                qT[:, h, qt * QT:(qt + 1) * QT],
                                start=True, stop=True)
                            if kc >= 4 * qt:
                                nc.vector.tensor_add(
                                    p_s, p_s, mask_sb[:, kc - 4 * qt, :])
                            expT = attnp.tile([128, QT], F32R, tag="expT")
                            nc.scalar.activation(expT, p_s, AF.Exp, scale=SCALE)
                            nc.tensor.matmul(p_den, ones, expT,
                                             start=(kc == 0), stop=(kc == nkc - 1))
                            nc.tensor.matmul(
                                p_o, v_sb[:, kc, h * HD:(h + 1) * HD], expT,
                                start=(kc == 0), stop=(kc == nkc - 1))
                        oT = attnp.tile([128, QT], F32R, tag="oT", bufs=4)
                        nc.vector.tensor_copy(oT, p_o)
                        o_qt.append(oT)
                        # den [1, 512] -> inv_den[:, h, 4qt:4qt+4] via DRAM bounce
                        den_sb = attnp.tile([1, QT], F32, tag="densb", bufs=2)
                        nc.vector.tensor_copy(den_sb, p_den)
                        bounce = dramp.tile([1, QT], F32, tag="bnc")
                        nc.gpsimd.dma_start(out=bounce, in_=den_sb)
                        den_cols = attnp.tile([128, 4], F32, tag="dencols", bufs=2)
                        nc.gpsimd.dma_start(
                            out=den_cols,
                            in_=bounce.rearrange("o (j p) -> (o p) j", p=128))
                        nc.vector.reciprocal(inv_den[:, h, 4 * qt:4 * qt + 4], den_cols)

                    # out-projection for the 4 L-tiles of this q-tile
                    for tt in range(4):
                        t = 4 * qt + tt
                        for ec in range(4):
                            p_f0 = ps_f.tile([128, 512], F32, tag="f0")
                            p_f1 = ps_f.tile([128, 512], F32, tag="f1")
                            nc.tensor.matmul(p_f0, o_qt[0][:, tt * LT:(tt + 1) * LT],
                                             wo_sb[:, 0, ec * 512:(ec + 1) * 512],
                                             start=True, stop=True)
                            nc.tensor.matmul(p_f1, o_qt[1][:, tt * LT:(tt + 1) * LT],
                                             wo_sb[:, 1, ec * 512:(ec + 1) * 512],
                                             start=True, stop=True)
                            y = resp.tile([128, 512], F32, tag="y")
                            if ec % 2 == 0:
                                nc.scalar.activation(y, p_f0, AF.Copy,
                                                     scale=inv_den[:, 0, t:t + 1])
                            else:
                                nc.vector.tensor_scalar_mul(y, p_f0,
                                                            inv_den[:, 0, t:t + 1])
                            nc.vector.scalar_tensor_tensor(
                                out=y, in0=p_f1, scalar=inv_den[:, 1, t:t + 1], in1=y,
                                op0=ALU.mult, op1=ALU.add)
                            nc.sync.dma_start(
                                out=out[t * LT:(t + 1) * LT, ec * 512:(ec + 1) * 512],
                                in_=y)
    nc.compile()
    return nc


_NC_CACHE = None


def _get_nc():
    global _NC_CACHE
    if _NC_CACHE is None:
        _NC_CACHE = build()
    return _NC_CACHE


def prep_inputs(x, w_qkv, w_out):
    """Host-side sharding/layout prep. Returns list of per-core input maps."""
    x2d = np.asarray(x, dtype=np.float32).reshape(L, D)
    w_qkv = np.asarray(w_qkv, dtype=np.float32)
    w_out = np.asarray(w_out, dtype=np.float32)

    # xt[t, c, p, l] = x2d[t*128 + l, c*128 + p]
    xt = np.ascontiguousarray(
        x2d.reshape(NT, LT, NHC, HC).transpose(0, 2, 3, 1))

    # rope coefficient tables
    inv_freq = 1.0 / (ROPE_BASE ** (np.arange(0, HD, 2, dtype=np.float64) / HD))
    pos = np.arange(L, dtype=np.float64)[:, None]
    ang = pos * inv_freq[None, :]                       # [L, 64]
    cos, sin = np.cos(ang), np.sin(ang)
    w1 = np.zeros((L, HD), dtype=np.float32)
    w2 = np.zeros((L, HD), dtype=np.float32)
    w1[:, 0::2] = -sin
    w1[:, 1::2] = cos
    w2[:, 0::2] = cos
    w2[:, 1::2] = sin
    w1 = np.ascontiguousarray(w1.reshape(NT, LT, HD))
    w2 = np.ascontiguousarray(w2.reshape(NT, LT, HD))

    # causal mask tiles for diagonal blocks
    i = np.arange(128)[:, None]
    j = np.arange(QT)[None, :]
    mask4 = np.stack(
        [np.where(i + r * 128 <= j, 0.0, NEG).astype(np.float32) for r in range(4)],
        axis=1)  # [128, 4, QT]

    ident = np.eye(128, dtype=np.float32)
    ones = np.ones((128, 1), dtype=np.float32)

    in_maps = []
    for c in range(N_CORES):
        h0 = HPC * c
        rows = []
        for part in range(3):  # q, k, v
            for hh in range(HPC):
                base = part * D + (h0 + hh) * HD
                rows.append(w_qkv[base:base + HD])
        w_c = np.concatenate(rows, axis=0)              # [768, D]
        wt = np.ascontiguousarray(w_c.T)                # [D, 768]
        wo = np.ascontiguousarray(
            w_out[:, h0 * HD:(h0 + HPC) * HD].T.reshape(HPC, HD, D))
        in_maps.append({
            "xt": xt, "wt": wt, "wo": wo, "w1": w1, "w2": w2,
            "mask4": mask4, "ident": ident, "ones": ones,
        })
    return in_maps


def kernel(x, w_qkv, w_out, mask, _trace=False):
    """Full MHA forward. Returns [1, L, D] float32."""
    nc = _get_nc()
    in_maps = prep_inputs(x, w_qkv, w_out)
    res = run_bass_kernel_spmd(nc, in_maps, core_ids=list(range(N_CORES)),
                               trace=_trace)
    acc = np.zeros((L, D), dtype=np.float32)
    for r in res.results:
        acc += r["out"]
    out = acc.reshape(1, L, D)
    if _trace:
        return out, res
    return out


# revision 6
# speedup vs baseline: 1.0797x; 1.0318x over previous
"""Trainium2 Bass kernel for 16-head causal MHA (RMSNorm+RoPE on q,k).

Tensor-parallel over heads: 8 cores x 2 heads each. Each core computes
qkv projection for its heads, norm+rope, causal attention, and a partial
out-projection; the host sums the 8 partial outputs.

Layout notes:
- L = 2048 seq, D = 2048 hidden, 16 heads x 128 head_dim.
- Scores are computed transposed [k, q] so softmax denominator comes from a
  ones-vector matmul and PV needs no transposes of the attention matrix.
- RoPE is computed into a de-interleaved [odd-half | even-half] d-order,
  identically for q and k (dot products unchanged); v / out_proj keep the
  natural d-order.
- exp() is computed without max-subtraction: post-RMSNorm |q.k|/sqrt(hd)
  <= sqrt(128), so exp is bounded by ~8.2e4. Masked (upper-triangular)
  score blocks are skipped entirely; diagonal blocks get an additive -1e9.
"""
import os
import numpy as np

import concourse.bacc as bacc
import concourse.mybir as mybir
import concourse.tile as tile
from concourse.ap import AP
from concourse.bass_utils import run_bass_kernel_spmd


def _bcast_mid(ap2d, n):
    """[128, X] -> [128, n, X] with step-0 middle dim."""
    return AP(tensor=ap2d.tensor, offset=ap2d.offset,
              ap=[list(ap2d.ap[0]), [0, n], list(ap2d.ap[1])])

F32 = mybir.dt.float32
F32R = mybir.dt.float32r
AF = mybir.ActivationFunctionType
ALU = mybir.AluOpType
AX = mybir.AxisListType

N_CORES = 8
L = 2048
D = 2048
HD = 128
N_HEAD = 16
HPC = N_HEAD // N_CORES  # heads per core = 2
LT = 128                 # L-tile rows
NT = L // LT             # 16 L-tiles
HC = 128                 # hid chunk
NHC = D // HC            # 16 hid chunks
QT = 512                 # q-tile width in attention
NQT = L // QT            # 4
EPS = 1e-5
ROPE_BASE = 10000.0
SCALE = 1.0 / float(np.sqrt(HD))
NEG = -1.0e9


def build():
    nc = bacc.Bacc("TRN2", target_bir_lowering=False, debug=False,
                   enable_asserts=False, num_devices=N_CORES)

    # Per-core external inputs (host-prepped layouts; see prep_inputs()).
    xt = nc.dram_tensor("xt", [NT, NHC, HC, LT], F32R, kind="ExternalInput")
    wt = nc.dram_tensor("wt", [D, 6 * HD], F32R, kind="ExternalInput")
    wo = nc.dram_tensor("wo", [HPC, HD, D], F32R, kind="ExternalInput")
    w1 = nc.dram_tensor("w1", [NT, LT, HD], F32, kind="ExternalInput")
    w2 = nc.dram_tensor("w2", [NT, LT, HD], F32, kind="ExternalInput")
    mask4 = nc.dram_tensor("mask4", [128, 4, QT], F32, kind="ExternalInput")
    ident_in = nc.dram_tensor("ident", [128, 128], F32R, kind="ExternalInput")
    ones_in = nc.dram_tensor("ones", [128, 1], F32R, kind="ExternalInput")

    out = nc.dram_tensor("out", [L, D], F32, kind="ExternalOutput")

    with tile.TileContext(nc) as tc:
        with (
            tc.tile_pool(name="const", bufs=1) as constp,
            tc.tile_pool(name="wpool", bufs=1) as wpool,
            tc.tile_pool(name="persist", bufs=1) as persist,
            tc.tile_pool(name="dram", bufs=2, space="DRAM") as dramp,
        ):
            # ---- constants ----
            ident = constp.tile([128, 128], F32R)
            nc.sync.dma_start(out=ident, in_=ident_in[:, :])
            ones = constp.tile([128, 1], F32R)
            nc.sync.dma_start(out=ones, in_=ones_in[:, :])
            eps_sb = constp.tile([128, 1], F32)
            nc.vector.memset(eps_sb, EPS)
            mask_sb = constp.tile([128, 4, QT], F32)
            nc.sync.dma_start(out=mask_sb, in_=mask4[:, :, :])
            w1_sb = constp.tile([128, NT, HD], F32)
            nc.sync.dma_start(out=w1_sb, in_=w1.rearrange("t p d -> p t d"))
            w2_sb = constp.tile([128, NT, HD], F32)
            nc.sync.dma_start(out=w2_sb, in_=w2.rearrange("t p d -> p t d"))

            # ---- weights resident ----
            w_sb = wpool.tile([128, NHC, 6 * HD], F32R)
            nc.sync.dma_start(out=w_sb, in_=wt.rearrange("(c p) e -> p c e", p=128))
            wo_sb = wpool.tile([128, HPC, D], F32R)
            nc.sync.dma_start(out=wo_sb, in_=wo.rearrange("h p e -> p h e"))

            # persistent activations
            v_sb = persist.tile([128, NT, HPC * HD], F32R)       # [L-part, t, v cols]
            qT = persist.tile([128, HPC, L], F32R)               # [d, h, L]
            kT = persist.tile([128, HPC, L], F32R)
            inv_den = persist.tile([128, HPC, NT], F32)

            # ================= Phase 1: QKV + norm + rope + transpose ======
            with (
                tc.tile_pool(name="xin", bufs=3) as xin,
                tc.tile_pool(name="qkv", bufs=3) as qkvp,
                tc.tile_pool(name="ps_qkv", bufs=2, space="PSUM") as ps_qkv,
                tc.tile_pool(name="ps_tr", bufs=2, space="PSUM") as ps_tr,
            ):
                for t in range(NT):
                    x_tile = xin.tile([128, NHC, LT], F32R, tag="x")
                    nc.sync.dma_start(out=x_tile, in_=xt[t].rearrange("c p l -> p c l"))

                    p_qk = ps_qkv.tile([128, 4 * HD], F32, tag="pqk")
                    p_v = ps_qkv.tile([128, HPC * HD], F32, tag="pv")
                    for c in range(NHC):
                        nc.tensor.matmul(p_qk, x_tile[:, c, :], w_sb[:, c, 0:4 * HD],
                                         start=(c == 0), stop=(c == NHC - 1))
                        nc.tensor.matmul(p_v, x_tile[:, c, :], w_sb[:, c, 4 * HD:6 * HD],
                                         start=(c == 0), stop=(c == NHC - 1))

                    # v -> persistent sbuf (natural layout)
                    nc.scalar.copy(v_sb[:, t, :], p_v)

                    # rms-norm scale: s = 1/sqrt(mean(x^2) + eps) per (L, seg)
                    sq = qkvp.tile([128, 4 * HD], F32, tag="sq")
                    nc.scalar.activation(sq, p_qk, AF.Square)
                    ssum = qkvp.tile([128, 4], F32, tag="ssum")
                    nc.vector.reduce_sum(ssum, sq.rearrange("p (g d) -> p g d", g=4),
                                         axis=AX.X)
                    nc.scalar.activation(ssum, ssum, AF.Sqrt, scale=1.0 / HD, bias=eps_sb)
                    s_val = qkvp.tile([128, 4], F32, tag="sval")
                    nc.vector.reciprocal(s_val, ssum)

                    # rope (batched over the 4 segments):
                    # qk_n = qk * s;  z = qk_n .* W1/W2;  pairwise-add -> halves
                    qk_n = qkvp.tile([128, 4 * HD], F32, tag="qkn")
                    nc.vector.tensor_mul(qk_n.rearrange("p (g d) -> p g d", g=4),
                                         p_qk.rearrange("p (g d) -> p g d", g=4),
                                         s_val.to_broadcast([128, 4, HD]))
                    roped = qkvp.tile([128, 4 * HD], F32R, tag="roped")
                    roped4 = roped.rearrange("p (g h x) -> p g h x", g=4, h=2)
                    for half, wtab in ((0, w1_sb), (1, w2_sb)):
                        z = qkvp.tile([128, 4 * HD], F32, tag="z")
                        nc.vector.tensor_mul(z.rearrange("p (g d) -> p g d", g=4),
                                             qk_n.rearrange("p (g d) -> p g d", g=4),
                                             _bcast_mid(wtab[:, t, :], 4))
                        with nc.allow_low_precision("2-elem rope pairs"):
                            nc.vector.reduce_sum(
                                roped4[:, :, half, :],
                                z.rearrange("p (g x two) -> p g x two", g=4, two=2),
                                axis=AX.X)

                    # transpose the 4 roped [128,128] blocks into qT/kT
                    for seg in range(4):
                        tgt = qT if seg < 2 else kT
                        h = seg % 2
                        p_tr = ps_tr.tile([128, 128], F32R, tag="ptr")
                        nc.tensor.transpose(
                            p_tr, roped[:, seg * HD:(seg + 1) * HD], ident)
                        nc.scalar.copy(tgt[:, h, t * LT:(t + 1) * LT], p_tr)

            # ====== Phase 2: attention + out-projection, interleaved per qt ==
            with (
                tc.tile_pool(name="attn", bufs=4) as attnp,
                tc.tile_pool(name="res", bufs=4) as resp,
                tc.tile_pool(name="ps_s", bufs=2, space="PSUM") as ps_s,
                tc.tile_pool(name="ps_o", bufs=2, space="PSUM") as ps_o,
                tc.tile_pool(name="ps_d", bufs=2, space="PSUM") as ps_d,
                tc.tile_pool(name="ps_f", bufs=1, space="PSUM") as ps_f,
            ):
                for qt in range(NQT):
                    o_qt = []
                    for h in range(HPC):
                        nkc = 4 * qt + 4
                        p_o = ps_o.tile([128, QT], F32, tag="po")
                        p_den = ps_d.tile([1, QT], F32, tag="pd")
                        for kc in range(nkc):
                            p_s = ps_s.tile([128, QT], F32, tag="ps")
                            nc.tensor.matmul(
                                p_s, kT[:, h, kc * 128:(kc + 1) * 128],
                                qT[:, h, qt * QT:(qt + 1) * QT],
                                start=True, stop=True)
                            if kc >= 4 * qt:
                                nc.vector.tensor_add(
                                    p_s, p_s, mask_sb[:, kc - 4 * qt, :])
                            expT = attnp.tile([128, QT], F32R, tag="expT")
                            nc.scalar.activation(expT, p_s, AF.Exp, scale=SCALE)
                            nc.tensor.matmul(p_den, ones, expT,
                                             start=(kc == 0), stop=(kc == nkc - 1))
                            nc.tensor.matmul(
                                p_o, v_sb[:, kc, h * HD:(h + 1) * HD], expT,
                                start=(kc == 0), stop=(kc == nkc - 1))
                        oT = attnp.tile([128, QT], F32R, tag="oT", bufs=4)
                        nc.scalar.copy(oT, p_o)
                        o_qt.append(oT)
                        # den [1, 512] -> inv_den[:, h, 4qt:4qt+4] via DRAM bounce
                        den_sb = attnp.tile([1, QT], F32, tag="densb", bufs=2)
                        nc.vector.tensor_copy(den_sb, p_den)
                        bounce = dramp.tile([1, QT], F32, tag="bnc")
                        nc.gpsimd.dma_start(out=bounce, in_=den_sb)
                        den_cols = attnp.tile([128, 4], F32, tag="dencols", bufs=2)
                        nc.gpsimd.dma_start(
                            out=den_cols,
                            in_=bounce.rearrange("o (j p) -> (o p) j", p=128))
                        nc.vector.reciprocal(inv_den[:, h, 4 * qt:4 * qt + 4], den_cols)

                    # out-projection for the 4 L-tiles of this q-tile
                    for tt in range(4):
                        t = 4 * qt + tt
                        for ec in range(4):
                            p_f0 = ps_f.tile([128, 512], F32, tag="f0")
                            p_f1 = ps_f.tile([128, 512], F32, tag="f1")
                            nc.tensor.matmul(p_f0, o_qt[0][:, tt * LT:(tt + 1) * LT],
                                             wo_sb[:, 0, ec * 512:(ec + 1) * 512],
                                             start=True, stop=True)
                            nc.tensor.matmul(p_f1, o_qt[1][:, tt * LT:(tt + 1) * LT],
                                             wo_sb[:, 1, ec * 512:(ec + 1) * 512],
                                             start=True, stop=True)
                            y = resp.tile([128, 512], F32, tag="y")
                            if ec % 2 == 0:
                                nc.scalar.activation(y, p_f0, AF.Copy,
                                                     scale=inv_den[:, 0, t:t + 1])
                            else:
                                nc.vector.tensor_scalar_mul(y, p_f0,
                                                            inv_den[:, 0, t:t + 1])
                            nc.vector.scalar_tensor_tensor(
                                out=y, in0=p_f1, scalar=inv_den[:, 1, t:t + 1], in1=y,
                                op0=ALU.mult, op1=ALU.add)
                            nc.sync.dma_start(
                                out=out[t * LT:(t + 1) * LT, ec * 512:(ec + 1) * 512],
                                in_=y)
    nc.compile()
    return nc


_NC_CACHE = None


def _get_nc():
    global _NC_CACHE
    if _NC_CACHE is None:
        _NC_CACHE = build()
    return _NC_CACHE


def prep_inputs(x, w_qkv, w_out):
    """Host-side sharding/layout prep. Returns list of per-core input maps."""
    x2d = np.asarray(x, dtype=np.float32).reshape(L, D)
    w_qkv = np.asarray(w_qkv, dtype=np.float32)
    w_out = np.asarray(w_out, dtype=np.float32)

    # xt[t, c, p, l] = x2d[t*128 + l, c*128 + p]
    xt = np.ascontiguousarray(
        x2d.reshape(NT, LT, NHC, HC).transpose(0, 2, 3, 1))

    # rope coefficient tables
    inv_freq = 1.0 / (ROPE_BASE ** (np.arange(0, HD, 2, dtype=np.float64) / HD))
    pos = np.arange(L, dtype=np.float64)[:, None]
    ang = pos * inv_freq[None, :]                       # [L, 64]
    cos, sin = np.cos(ang), np.sin(ang)
    w1 = np.zeros((L, HD), dtype=np.float32)
    w2 = np.zeros((L, HD), dtype=np.float32)
    w1[:, 0::2] = -sin
    w1[:, 1::2] = cos
    w2[:, 0::2] = cos
    w2[:, 1::2] = sin
    w1 = np.ascontiguousarray(w1.reshape(NT, LT, HD))
    w2 = np.ascontiguousarray(w2.reshape(NT, LT, HD))

    # causal mask tiles for diagonal blocks
    i = np.arange(128)[:, None]
    j = np.arange(QT)[None, :]
    mask4 = np.stack(
        [np.where(i + r * 128 <= j, 0.0, NEG).astype(np.float32) for r in range(4)],
        axis=1)  # [128, 4, QT]

    ident = np.eye(128, dtype=np.float32)
    ones = np.ones((128, 1), dtype=np.float32)

    in_maps = []
    for c in range(N_CORES):
        h0 = HPC * c
        rows = []
        for part in range(3):  # q, k, v
            for hh in range(HPC):
                base = part * D + (h0 + hh) * HD
                rows.append(w_qkv[base:base + HD])
        w_c = np.concatenate(rows, axis=0)              # [768, D]
        wt = np.ascontiguousarray(w_c.T)                # [D, 768]
        wo = np.ascontiguousarray(
            w_out[:, h0 * HD:(h0 + HPC) * HD].T.reshape(HPC, HD, D))
        in_maps.append({
            "xt": xt, "wt": wt, "wo": wo, "w1": w1, "w2": w2,
            "mask4": mask4, "ident": ident, "ones": ones,
        })
    return in_maps


def kernel(x, w_qkv, w_out, mask, _trace=False):
    """Full MHA forward. Returns [1, L, D] float32."""
    nc = _get_nc()
    in_maps = prep_inputs(x, w_qkv, w_out)
    res = run_bass_kernel_spmd(nc, in_maps, core_ids=list(range(N_CORES)),
                               trace=_trace)
    acc = np.zeros((L, D), dtype=np.float32)
    for r in res.results:
        acc += r["out"]
    out = acc.reshape(1, L, D)
    if _trace:
        return out, res
    return out


# revision 7
# speedup vs baseline: 1.1931x; 1.1051x over previous
"""Trainium2 Bass kernel for 16-head causal MHA (RMSNorm+RoPE on q,k).

Tensor-parallel over heads: 8 cores x 2 heads each. Each core computes
qkv projection for its heads, norm+rope, causal attention, and a partial
out-projection; the host sums the 8 partial outputs.

Layout notes:
- L = 2048 seq, D = 2048 hidden, 16 heads x 128 head_dim.
- Scores are computed transposed [k, q] so softmax denominator comes from a
  ones-vector matmul and PV needs no transposes of the attention matrix.
- RoPE is computed into a de-interleaved [odd-half | even-half] d-order,
  identically for q and k (dot products unchanged); v / out_proj keep the
  natural d-order.
- exp() is computed without max-subtraction: post-RMSNorm |q.k|/sqrt(hd)
  <= sqrt(128), so exp is bounded by ~8.2e4. Masked (upper-triangular)
  score blocks are skipped entirely; diagonal blocks get an additive -1e9.
"""
import os
import numpy as np

import concourse.bacc as bacc
import concourse.mybir as mybir
import concourse.tile as tile
from concourse.ap import AP
from concourse.bass_utils import run_bass_kernel_spmd


def _bcast_mid(ap2d, n):
    """[128, X] -> [128, n, X] with step-0 middle dim."""
    return AP(tensor=ap2d.tensor, offset=ap2d.offset,
              ap=[list(ap2d.ap[0]), [0, n], list(ap2d.ap[1])])

F32 = mybir.dt.float32
F32R = mybir.dt.float32r
AF = mybir.ActivationFunctionType
ALU = mybir.AluOpType
AX = mybir.AxisListType

N_CORES = 8
L = 2048
D = 2048
HD = 128
N_HEAD = 16
HPC = N_HEAD // N_CORES  # heads per core = 2
LT = 128                 # L-tile rows
NT = L // LT             # 16 L-tiles
HC = 128                 # hid chunk
NHC = D // HC            # 16 hid chunks
QT = 512                 # q-tile width in attention
NQT = L // QT            # 4
EPS = 1e-5
ROPE_BASE = 10000.0
SCALE = 1.0 / float(np.sqrt(HD))
NEG = -1.0e9


def build():
    nc = bacc.Bacc("TRN2", target_bir_lowering=False, debug=False,
                   enable_asserts=False, num_devices=N_CORES)

    # Per-core external inputs (host-prepped layouts; see prep_inputs()).
    xt = nc.dram_tensor("xt", [NT, NHC, HC, LT], F32R, kind="ExternalInput")
    wt = nc.dram_tensor("wt", [D, 6 * HD], F32R, kind="ExternalInput")
    wo = nc.dram_tensor("wo", [HPC, HD, D], F32R, kind="ExternalInput")
    w1 = nc.dram_tensor("w1", [NT, LT, HD], F32, kind="ExternalInput")
    w2 = nc.dram_tensor("w2", [NT, LT, HD], F32, kind="ExternalInput")
    mask4 = nc.dram_tensor("mask4", [128, 128], F32, kind="ExternalInput")
    ident_in = nc.dram_tensor("ident", [128, 128], F32R, kind="ExternalInput")
    ones_in = nc.dram_tensor("ones", [128, 1], F32R, kind="ExternalInput")

    out = nc.dram_tensor("out", [L, D], F32, kind="ExternalOutput")

    with tile.TileContext(nc) as tc:
        with (
            tc.tile_pool(name="const", bufs=1) as constp,
            tc.tile_pool(name="wpool", bufs=1) as wpool,
            tc.tile_pool(name="persist", bufs=1) as persist,
            tc.tile_pool(name="dram", bufs=2, space="DRAM") as dramp,
        ):
            # ---- constants ----
            ident = constp.tile([128, 128], F32R)
            nc.sync.dma_start(out=ident, in_=ident_in[:, :])
            ones = constp.tile([128, 1], F32R)
            nc.sync.dma_start(out=ones, in_=ones_in[:, :])
            eps_sb = constp.tile([128, 1], F32)
            nc.vector.memset(eps_sb, EPS)
            mask_sb = constp.tile([128, 128], F32)
            nc.sync.dma_start(out=mask_sb, in_=mask4[:, :])
            w1_sb = constp.tile([128, NT, HD], F32)
            nc.sync.dma_start(out=w1_sb, in_=w1.rearrange("t p d -> p t d"))
            w2_sb = constp.tile([128, NT, HD], F32)
            nc.sync.dma_start(out=w2_sb, in_=w2.rearrange("t p d -> p t d"))

            # ---- weights resident ----
            w_sb = wpool.tile([128, NHC, 6 * HD], F32R)
            for c in range(NHC):
                nc.sync.dma_start(out=w_sb[:, c, :],
                                  in_=wt[c * 128:(c + 1) * 128, :])
            wo_sb = wpool.tile([128, HPC, D], F32R)
            nc.sync.dma_start(out=wo_sb, in_=wo.rearrange("h p e -> p h e"))

            # persistent activations
            v_sb = persist.tile([128, NT, HPC * HD], F32R)       # [L-part, t, v cols]
            qT = persist.tile([128, HPC, L], F32R)               # [d, h, L]
            kT = persist.tile([128, HPC, L], F32R)
            inv_den = persist.tile([128, HPC, NT], F32)

            # ================= Phase 1: QKV + norm + rope + transpose ======
            with (
                tc.tile_pool(name="xin", bufs=3) as xin,
                tc.tile_pool(name="qkv", bufs=3) as qkvp,
                tc.tile_pool(name="ps_qkv", bufs=2, space="PSUM") as ps_qkv,
                tc.tile_pool(name="ps_tr", bufs=2, space="PSUM") as ps_tr,
            ):
                for t in range(NT):
                    x_tile = xin.tile([128, NHC, LT], F32R, tag="x")
                    nc.sync.dma_start(out=x_tile, in_=xt[t].rearrange("c p l -> p c l"))

                    p_qk = ps_qkv.tile([128, 4 * HD], F32, tag="pqk")
                    p_v = ps_qkv.tile([128, HPC * HD], F32, tag="pv")
                    for c in range(NHC):
                        nc.tensor.matmul(p_qk, x_tile[:, c, :], w_sb[:, c, 0:4 * HD],
                                         start=(c == 0), stop=(c == NHC - 1))
                        nc.tensor.matmul(p_v, x_tile[:, c, :], w_sb[:, c, 4 * HD:6 * HD],
                                         start=(c == 0), stop=(c == NHC - 1))

                    # v -> persistent sbuf (natural layout)
                    nc.scalar.copy(v_sb[:, t, :], p_v)

                    # rms-norm scale: s = 1/sqrt(mean(x^2) + eps) per (L, seg)
                    sq = qkvp.tile([128, 4 * HD], F32, tag="sq")
                    nc.scalar.activation(sq, p_qk, AF.Square)
                    ssum = qkvp.tile([128, 4], F32, tag="ssum")
                    nc.vector.reduce_sum(ssum, sq.rearrange("p (g d) -> p g d", g=4),
                                         axis=AX.X)
                    nc.scalar.activation(ssum, ssum, AF.Sqrt, scale=1.0 / HD, bias=eps_sb)
                    s_val = qkvp.tile([128, 4], F32, tag="sval")
                    nc.vector.reciprocal(s_val, ssum)

                    # rope (batched over the 4 segments):
                    # qk_n = qk * s;  z = qk_n .* W1/W2;  pairwise-add -> halves
                    qk_n = qkvp.tile([128, 4 * HD], F32, tag="qkn")
                    nc.vector.tensor_mul(qk_n.rearrange("p (g d) -> p g d", g=4),
                                         p_qk.rearrange("p (g d) -> p g d", g=4),
                                         s_val.to_broadcast([128, 4, HD]))
                    roped = qkvp.tile([128, 4 * HD], F32R, tag="roped")
                    roped4 = roped.rearrange("p (g h x) -> p g h x", g=4, h=2)
                    for half, wtab in ((0, w1_sb), (1, w2_sb)):
                        z = qkvp.tile([128, 4 * HD], F32, tag="z")
                        nc.vector.tensor_mul(z.rearrange("p (g d) -> p g d", g=4),
                                             qk_n.rearrange("p (g d) -> p g d", g=4),
                                             _bcast_mid(wtab[:, t, :], 4))
                        with nc.allow_low_precision("2-elem rope pairs"):
                            nc.vector.reduce_sum(
                                roped4[:, :, half, :],
                                z.rearrange("p (g x two) -> p g x two", g=4, two=2),
                                axis=AX.X)

                    # transpose the 4 roped [128,128] blocks into qT/kT
                    for seg in range(4):
                        tgt = qT if seg < 2 else kT
                        h = seg % 2
                        p_tr = ps_tr.tile([128, 128], F32R, tag="ptr")
                        nc.tensor.transpose(
                            p_tr, roped[:, seg * HD:(seg + 1) * HD], ident)
                        nc.scalar.copy(tgt[:, h, t * LT:(t + 1) * LT], p_tr)

            # ====== Phase 2: attention + out-projection, interleaved per qt ==
            with (
                tc.tile_pool(name="attn", bufs=4) as attnp,
                tc.tile_pool(name="res", bufs=4) as resp,
                tc.tile_pool(name="ps_s", bufs=3, space="PSUM") as ps_s,
                tc.tile_pool(name="ps_o", bufs=2, space="PSUM") as ps_o,
                tc.tile_pool(name="ps_d", bufs=1, space="PSUM") as ps_d,
                tc.tile_pool(name="ps_f", bufs=1, space="PSUM") as ps_f,
            ):
                for qt in range(NQT):
                    o_qt = []
                    for h in range(HPC):
                        nkc = 4 * qt + 4
                        p_o = ps_o.tile([128, QT], F32, tag="po")
                        p_den = ps_d.tile([1, QT], F32, tag="pd")
                        for kc in range(nkc):
                            diag = kc >= 4 * qt
                            q0 = (kc - 4 * qt) * 128 if diag else 0
                            p_s = ps_s.tile([128, QT], F32, tag="ps")
                            nc.tensor.matmul(
                                p_s[:, q0:QT], kT[:, h, kc * 128:(kc + 1) * 128],
                                qT[:, h, qt * QT + q0:(qt + 1) * QT],
                                start=True, stop=True)
                            if diag:
                                nc.vector.tensor_add(
                                    p_s[:, q0:q0 + 128], p_s[:, q0:q0 + 128],
                                    mask_sb)
                            expT = attnp.tile([128, QT], F32R, tag="expT", bufs=6)
                            nc.scalar.activation(expT[:, q0:QT], p_s[:, q0:QT],
                                                 AF.Exp, scale=SCALE)
                            nc.tensor.matmul(p_den[:, q0:QT], ones, expT[:, q0:QT],
                                             start=(kc == 0), stop=(kc == nkc - 1))
                            nc.tensor.matmul(
                                p_o[:, q0:QT], v_sb[:, kc, h * HD:(h + 1) * HD],
                                expT[:, q0:QT],
                                start=(kc == 0), stop=(kc == nkc - 1))
                        oT = attnp.tile([128, QT], F32R, tag="oT", bufs=4)
                        nc.scalar.copy(oT, p_o)
                        o_qt.append(oT)
                        # den [1, 512] -> inv_den[:, h, 4qt:4qt+4] via DRAM bounce
                        den_sb = attnp.tile([1, QT], F32, tag="densb", bufs=2)
                        nc.vector.tensor_copy(den_sb, p_den)
                        bounce = dramp.tile([1, QT], F32, tag="bnc")
                        nc.gpsimd.dma_start(out=bounce, in_=den_sb)
                        den_cols = attnp.tile([128, 4], F32, tag="dencols", bufs=2)
                        nc.gpsimd.dma_start(
                            out=den_cols,
                            in_=bounce.rearrange("o (j p) -> (o p) j", p=128))
                        nc.vector.reciprocal(inv_den[:, h, 4 * qt:4 * qt + 4], den_cols)

                    # out-projection for the 4 L-tiles of this q-tile
                    for tt in range(4):
                        t = 4 * qt + tt
                        for ec in range(4):
                            p_f0 = ps_f.tile([128, 512], F32, tag="f0")
                            p_f1 = ps_f.tile([128, 512], F32, tag="f1")
                            nc.tensor.matmul(p_f0, o_qt[0][:, tt * LT:(tt + 1) * LT],
                                             wo_sb[:, 0, ec * 512:(ec + 1) * 512],
                                             start=True, stop=True)
                            nc.tensor.matmul(p_f1, o_qt[1][:, tt * LT:(tt + 1) * LT],
                                             wo_sb[:, 1, ec * 512:(ec + 1) * 512],
                                             start=True, stop=True)
                            y = resp.tile([128, 512], F32, tag="y")
                            if ec % 2 == 0:
                                nc.scalar.activation(y, p_f0, AF.Copy,
                                                     scale=inv_den[:, 0, t:t + 1])
                            else:
                                nc.vector.tensor_scalar_mul(y, p_f0,
                                                            inv_den[:, 0, t:t + 1])
                            nc.vector.scalar_tensor_tensor(
                                out=y, in0=p_f1, scalar=inv_den[:, 1, t:t + 1], in1=y,
                                op0=ALU.mult, op1=ALU.add)
                            nc.sync.dma_start(
                                out=out[t * LT:(t + 1) * LT, ec * 512:(ec + 1) * 512],
                                in_=y)
    nc.compile()
    return nc


_NC_CACHE = None


def _get_nc():
    global _NC_CACHE
    if _NC_CACHE is None:
        _NC_CACHE = build()
    return _NC_CACHE


def prep_inputs(x, w_qkv, w_out):
    """Host-side sharding/layout prep. Returns list of per-core input maps."""
    x2d = np.asarray(x, dtype=np.float32).reshape(L, D)
    w_qkv = np.asarray(w_qkv, dtype=np.float32)
    w_out = np.asarray(w_out, dtype=np.float32)

    # xt[t, c, p, l] = x2d[t*128 + l, c*128 + p]
    xt = np.ascontiguousarray(
        x2d.reshape(NT, LT, NHC, HC).transpose(0, 2, 3, 1))

    # rope coefficient tables
    inv_freq = 1.0 / (ROPE_BASE ** (np.arange(0, HD, 2, dtype=np.float64) / HD))
    pos = np.arange(L, dtype=np.float64)[:, None]
    ang = pos * inv_freq[None, :]                       # [L, 64]
    cos, sin = np.cos(ang), np.sin(ang)
    w1 = np.zeros((L, HD), dtype=np.float32)
    w2 = np.zeros((L, HD), dtype=np.float32)
    w1[:, 0::2] = -sin
    w1[:, 1::2] = cos
    w2[:, 0::2] = cos
    w2[:, 1::2] = sin
    w1 = np.ascontiguousarray(w1.reshape(NT, LT, HD))
    w2 = np.ascontiguousarray(w2.reshape(NT, LT, HD))

    # causal mask tiles for diagonal blocks
    i = np.arange(128)[:, None]
    j = np.arange(128)[None, :]
    mask4 = np.where(i <= j, 0.0, NEG).astype(np.float32)  # [128, 128] triangular

    ident = np.eye(128, dtype=np.float32)
    ones = np.ones((128, 1), dtype=np.float32)

    in_maps = []
    for c in range(N_CORES):
        h0 = HPC * c
        rows = []
        for part in range(3):  # q, k, v
            for hh in range(HPC):
                base = part * D + (h0 + hh) * HD
                rows.append(w_qkv[base:base + HD])
        w_c = np.concatenate(rows, axis=0)              # [768, D]
        wt = np.ascontiguousarray(w_c.T)                # [D, 768]
        wo = np.ascontiguousarray(
            w_out[:, h0 * HD:(h0 + HPC) * HD].T.reshape(HPC, HD, D))
        in_maps.append({
            "xt": xt, "wt": wt, "wo": wo, "w1": w1, "w2": w2,
            "mask4": mask4, "ident": ident, "ones": ones,
        })
    return in_maps


def kernel(x, w_qkv, w_out, mask, _trace=False):
    """Full MHA forward. Returns [1, L, D] float32."""
    nc = _get_nc()
    in_maps = prep_inputs(x, w_qkv, w_out)
    res = run_bass_kernel_spmd(nc, in_maps, core_ids=list(range(N_CORES)),
                               trace=_trace)
    acc = np.zeros((L, D), dtype=np.float32)
    for r in res.results:
        acc += r["out"]
    out = acc.reshape(1, L, D)
    if _trace:
        return out, res
    return out


# revision 8
# speedup vs baseline: 1.2776x; 1.0707x over previous
"""Trainium2 Bass kernel for 16-head causal MHA (RMSNorm+RoPE on q,k).

Tensor-parallel over heads: 8 cores x 2 heads each. Each core computes
qkv projection for its heads, norm+rope, causal attention, and a partial
out-projection; the host sums the 8 partial outputs.

Layout notes:
- L = 2048 seq, D = 2048 hidden, 16 heads x 128 head_dim.
- Scores are computed transposed [k, q] so softmax denominator comes from a
  ones-vector matmul and PV needs no transposes of the attention matrix.
- RoPE is computed into a de-interleaved [odd-half | even-half] d-order,
  identically for q and k (dot products unchanged); v / out_proj keep the
  natural d-order.
- exp() is computed without max-subtraction: post-RMSNorm |q.k|/sqrt(hd)
  <= sqrt(128), so exp is bounded by ~8.2e4. Masked (upper-triangular)
  score blocks are skipped entirely; diagonal blocks get an additive -1e9.
"""
import os
import ml_dtypes
import numpy as np

import concourse.bacc as bacc
import concourse.mybir as mybir
import concourse.tile as tile
from concourse.ap import AP
from concourse.bass_utils import run_bass_kernel_spmd


def _bcast_mid(ap2d, n):
    """[128, X] -> [128, n, X] with step-0 middle dim."""
    return AP(tensor=ap2d.tensor, offset=ap2d.offset,
              ap=[list(ap2d.ap[0]), [0, n], list(ap2d.ap[1])])

F32 = mybir.dt.float32
F32R = mybir.dt.float32r
BF16 = mybir.dt.bfloat16
# working dtype for matmul operands: "bf16" or "f32r"
WDTYPE = os.environ.get("MHA_WDTYPE", "bf16")
WDT = BF16 if WDTYPE == "bf16" else F32R
AF = mybir.ActivationFunctionType
ALU = mybir.AluOpType
AX = mybir.AxisListType

N_CORES = 8
L = 2048
D = 2048
HD = 128
N_HEAD = 16
HPC = N_HEAD // N_CORES  # heads per core = 2
LT = 128                 # L-tile rows
NT = L // LT             # 16 L-tiles
HC = 128                 # hid chunk
NHC = D // HC            # 16 hid chunks
QT = 512                 # q-tile width in attention
NQT = L // QT            # 4
EPS = 1e-5
ROPE_BASE = 10000.0
SCALE = 1.0 / float(np.sqrt(HD))
NEG = -1.0e9


def build():
    nc = bacc.Bacc("TRN2", target_bir_lowering=False, debug=False,
                   enable_asserts=False, num_devices=N_CORES)

    # Per-core external inputs (host-prepped layouts; see prep_inputs()).
    xt = nc.dram_tensor("xt", [NT, NHC, HC, LT], WDT, kind="ExternalInput")
    wt = nc.dram_tensor("wt", [D, 6 * HD], WDT, kind="ExternalInput")
    wo = nc.dram_tensor("wo", [HPC, HD, D], WDT, kind="ExternalInput")
    w1 = nc.dram_tensor("w1", [NT, LT, HD], F32, kind="ExternalInput")
    w2 = nc.dram_tensor("w2", [NT, LT, HD], F32, kind="ExternalInput")
    mask4 = nc.dram_tensor("mask4", [128, 128], F32, kind="ExternalInput")
    ident_in = nc.dram_tensor("ident", [128, 128], WDT, kind="ExternalInput")
    ones_in = nc.dram_tensor("ones", [128, 1], WDT, kind="ExternalInput")

    out = nc.dram_tensor("out", [L, D], F32, kind="ExternalOutput")

    with tile.TileContext(nc) as tc:
        with (
            tc.tile_pool(name="const", bufs=1) as constp,
            tc.tile_pool(name="wpool", bufs=1) as wpool,
            tc.tile_pool(name="persist", bufs=1) as persist,
            tc.tile_pool(name="dram", bufs=2, space="DRAM") as dramp,
        ):
            # ---- constants ----
            ident = constp.tile([128, 128], WDT)
            nc.sync.dma_start(out=ident, in_=ident_in[:, :])
            ones = constp.tile([128, 1], WDT)
            nc.sync.dma_start(out=ones, in_=ones_in[:, :])
            eps_sb = constp.tile([128, 1], F32)
            nc.vector.memset(eps_sb, EPS)
            mask_sb = constp.tile([128, 128], F32)
            nc.sync.dma_start(out=mask_sb, in_=mask4[:, :])
            w1_sb = constp.tile([128, NT, HD], F32)
            nc.sync.dma_start(out=w1_sb, in_=w1.rearrange("t p d -> p t d"))
            w2_sb = constp.tile([128, NT, HD], F32)
            nc.sync.dma_start(out=w2_sb, in_=w2.rearrange("t p d -> p t d"))

            # ---- weights resident ----
            w_sb = wpool.tile([128, NHC, 6 * HD], WDT)
            for c in range(NHC):
                nc.sync.dma_start(out=w_sb[:, c, :],
                                  in_=wt[c * 128:(c + 1) * 128, :])
            wo_sb = wpool.tile([128, HPC, D], WDT)
            nc.sync.dma_start(out=wo_sb, in_=wo.rearrange("h p e -> p h e"))

            # persistent activations
            v_sb = persist.tile([128, NT, HPC * HD], WDT)       # [L-part, t, v cols]
            qT = persist.tile([128, HPC, L], WDT)               # [d, h, L]
            kT = persist.tile([128, HPC, L], WDT)
            inv_den = persist.tile([128, HPC, NT], F32)

            # ================= Phase 1: QKV + norm + rope + transpose ======
            with (
                tc.tile_pool(name="xin", bufs=3) as xin,
                tc.tile_pool(name="qkv", bufs=3) as qkvp,
                tc.tile_pool(name="ps_qkv", bufs=2, space="PSUM") as ps_qkv,
                tc.tile_pool(name="ps_tr", bufs=2, space="PSUM") as ps_tr,
            ):
                for t in range(NT):
                    x_tile = xin.tile([128, NHC, LT], WDT, tag="x")
                    nc.sync.dma_start(out=x_tile, in_=xt[t].rearrange("c p l -> p c l"))

                    p_qk = ps_qkv.tile([128, 4 * HD], F32, tag="pqk")
                    p_v = ps_qkv.tile([128, HPC * HD], F32, tag="pv")
                    for c in range(NHC):
                        nc.tensor.matmul(p_qk, x_tile[:, c, :], w_sb[:, c, 0:4 * HD],
                                         start=(c == 0), stop=(c == NHC - 1))
                        nc.tensor.matmul(p_v, x_tile[:, c, :], w_sb[:, c, 4 * HD:6 * HD],
                                         start=(c == 0), stop=(c == NHC - 1))

                    # v -> persistent sbuf (natural layout)
                    nc.scalar.copy(v_sb[:, t, :], p_v)

                    # rms-norm scale: s = 1/sqrt(mean(x^2) + eps) per (L, seg)
                    sq = qkvp.tile([128, 4 * HD], F32, tag="sq")
                    nc.scalar.activation(sq, p_qk, AF.Square)
                    ssum = qkvp.tile([128, 4], F32, tag="ssum")
                    nc.vector.reduce_sum(ssum, sq.rearrange("p (g d) -> p g d", g=4),
                                         axis=AX.X)
                    nc.scalar.activation(ssum, ssum, AF.Sqrt, scale=1.0 / HD, bias=eps_sb)
                    s_val = qkvp.tile([128, 4], F32, tag="sval")
                    nc.vector.reciprocal(s_val, ssum)

                    # rope (batched over the 4 segments):
                    # qk_n = qk * s;  z = qk_n .* W1/W2;  pairwise-add -> halves
                    qk_n = qkvp.tile([128, 4 * HD], F32, tag="qkn")
                    nc.vector.tensor_mul(qk_n.rearrange("p (g d) -> p g d", g=4),
                                         p_qk.rearrange("p (g d) -> p g d", g=4),
                                         s_val.to_broadcast([128, 4, HD]))
                    roped = qkvp.tile([128, 4 * HD], WDT, tag="roped")
                    roped4 = roped.rearrange("p (g h x) -> p g h x", g=4, h=2)
                    for half, wtab in ((0, w1_sb), (1, w2_sb)):
                        z = qkvp.tile([128, 4 * HD], F32, tag="z")
                        nc.vector.tensor_mul(z.rearrange("p (g d) -> p g d", g=4),
                                             qk_n.rearrange("p (g d) -> p g d", g=4),
                                             _bcast_mid(wtab[:, t, :], 4))
                        with nc.allow_low_precision("2-elem rope pairs"):
                            nc.vector.reduce_sum(
                                roped4[:, :, half, :],
                                z.rearrange("p (g x two) -> p g x two", g=4, two=2),
                                axis=AX.X)

                    # transpose the 4 roped [128,128] blocks into qT/kT
                    for seg in range(4):
                        tgt = qT if seg < 2 else kT
                        h = seg % 2
                        p_tr = ps_tr.tile([128, 128], WDT, tag="ptr")
                        nc.tensor.transpose(
                            p_tr, roped[:, seg * HD:(seg + 1) * HD], ident)
                        nc.scalar.copy(tgt[:, h, t * LT:(t + 1) * LT], p_tr)

            # ====== Phase 2: attention + out-projection, interleaved per qt ==
            with (
                tc.tile_pool(name="attn", bufs=4) as attnp,
                tc.tile_pool(name="res", bufs=4) as resp,
                tc.tile_pool(name="ps_s", bufs=3, space="PSUM") as ps_s,
                tc.tile_pool(name="ps_o", bufs=2, space="PSUM") as ps_o,
                tc.tile_pool(name="ps_d", bufs=1, space="PSUM") as ps_d,
                tc.tile_pool(name="ps_f", bufs=1, space="PSUM") as ps_f,
            ):
                for qt in range(NQT):
                    o_qt = []
                    for h in range(HPC):
                        nkc = 4 * qt + 4
                        p_o = ps_o.tile([128, QT], F32, tag="po")
                        p_den = ps_d.tile([1, QT], F32, tag="pd")
                        for kc in range(nkc):
                            diag = kc >= 4 * qt
                            q0 = (kc - 4 * qt) * 128 if diag else 0
                            p_s = ps_s.tile([128, QT], F32, tag="ps")
                            nc.tensor.matmul(
                                p_s[:, q0:QT], kT[:, h, kc * 128:(kc + 1) * 128],
                                qT[:, h, qt * QT + q0:(qt + 1) * QT],
                                start=True, stop=True)
                            if diag:
                                nc.vector.tensor_add(
                                    p_s[:, q0:q0 + 128], p_s[:, q0:q0 + 128],
                                    mask_sb)
                            expT = attnp.tile([128, QT], WDT, tag="expT", bufs=6)
                            nc.scalar.activation(expT[:, q0:QT], p_s[:, q0:QT],
                                                 AF.Exp, scale=SCALE)
                            nc.tensor.matmul(p_den[:, q0:QT], ones, expT[:, q0:QT],
                                             start=(kc == 0), stop=(kc == nkc - 1))
                            nc.tensor.matmul(
                                p_o[:, q0:QT], v_sb[:, kc, h * HD:(h + 1) * HD],
                                expT[:, q0:QT],
                                start=(kc == 0), stop=(kc == nkc - 1))
                        oT = attnp.tile([128, QT], WDT, tag="oT", bufs=4)
                        nc.scalar.copy(oT, p_o)
                        o_qt.append(oT)
                        # den [1, 512] -> inv_den[:, h, 4qt:4qt+4] via DRAM bounce
                        den_sb = attnp.tile([1, QT], F32, tag="densb", bufs=2)
                        nc.vector.tensor_copy(den_sb, p_den)
                        bounce = dramp.tile([1, QT], F32, tag="bnc")
                        nc.gpsimd.dma_start(out=bounce, in_=den_sb)
                        den_cols = attnp.tile([128, 4], F32, tag="dencols", bufs=2)
                        nc.gpsimd.dma_start(
                            out=den_cols,
                            in_=bounce.rearrange("o (j p) -> (o p) j", p=128))
                        nc.vector.reciprocal(inv_den[:, h, 4 * qt:4 * qt + 4], den_cols)

                    # out-projection for the 4 L-tiles of this q-tile
                    for tt in range(4):
                        t = 4 * qt + tt
                        for ec in range(4):
                            p_f0 = ps_f.tile([128, 512], F32, tag="f0")
                            p_f1 = ps_f.tile([128, 512], F32, tag="f1")
                            nc.tensor.matmul(p_f0, o_qt[0][:, tt * LT:(tt + 1) * LT],
                                             wo_sb[:, 0, ec * 512:(ec + 1) * 512],
                                             start=True, stop=True)
                            nc.tensor.matmul(p_f1, o_qt[1][:, tt * LT:(tt + 1) * LT],
                                             wo_sb[:, 1, ec * 512:(ec + 1) * 512],
                                             start=True, stop=True)
                            y = resp.tile([128, 512], F32, tag="y")
                            if ec % 2 == 0:
                                nc.scalar.activation(y, p_f0, AF.Copy,
                                                     scale=inv_den[:, 0, t:t + 1])
                            else:
                                nc.vector.tensor_scalar_mul(y, p_f0,
                                                            inv_den[:, 0, t:t + 1])
                            nc.vector.scalar_tensor_tensor(
                                out=y, in0=p_f1, scalar=inv_den[:, 1, t:t + 1], in1=y,
                                op0=ALU.mult, op1=ALU.add)
                            nc.sync.dma_start(
                                out=out[t * LT:(t + 1) * LT, ec * 512:(ec + 1) * 512],
                                in_=y)
    nc.compile()
    return nc


_NC_CACHE = None


def _get_nc():
    global _NC_CACHE
    if _NC_CACHE is None:
        _NC_CACHE = build()
    return _NC_CACHE


def prep_inputs(x, w_qkv, w_out):
    """Host-side sharding/layout prep. Returns list of per-core input maps."""
    wnp = ml_dtypes.bfloat16 if WDTYPE == "bf16" else np.float32
    x2d = np.asarray(x, dtype=np.float32).reshape(L, D)
    w_qkv = np.asarray(w_qkv, dtype=np.float32)
    w_out = np.asarray(w_out, dtype=np.float32)

    # xt[t, c, p, l] = x2d[t*128 + l, c*128 + p]
    xt = np.ascontiguousarray(
        x2d.reshape(NT, LT, NHC, HC).transpose(0, 2, 3, 1)).astype(wnp)

    # rope coefficient tables
    inv_freq = 1.0 / (ROPE_BASE ** (np.arange(0, HD, 2, dtype=np.float64) / HD))
    pos = np.arange(L, dtype=np.float64)[:, None]
    ang = pos * inv_freq[None, :]                       # [L, 64]
    cos, sin = np.cos(ang), np.sin(ang)
    w1 = np.zeros((L, HD), dtype=np.float32)
    w2 = np.zeros((L, HD), dtype=np.float32)
    w1[:, 0::2] = -sin
    w1[:, 1::2] = cos
    w2[:, 0::2] = cos
    w2[:, 1::2] = sin
    w1 = np.ascontiguousarray(w1.reshape(NT, LT, HD))
    w2 = np.ascontiguousarray(w2.reshape(NT, LT, HD))

    # causal mask tiles for diagonal blocks
    i = np.arange(128)[:, None]
    j = np.arange(128)[None, :]
    mask4 = np.where(i <= j, 0.0, NEG).astype(np.float32)  # [128, 128] triangular

    ident = np.eye(128, dtype=np.float32).astype(wnp)
    ones = np.ones((128, 1), dtype=np.float32).astype(wnp)

    in_maps = []
    for c in range(N_CORES):
        h0 = HPC * c
        rows = []
        for part in range(3):  # q, k, v
            for hh in range(HPC):
                base = part * D + (h0 + hh) * HD
                rows.append(w_qkv[base:base + HD])
        w_c = np.concatenate(rows, axis=0)              # [768, D]
        wt = np.ascontiguousarray(w_c.T).astype(wnp)    # [D, 768]
        wo = np.ascontiguousarray(
            w_out[:, h0 * HD:(h0 + HPC) * HD].T.reshape(HPC, HD, D)).astype(wnp)
        in_maps.append({
            "xt": xt, "wt": wt, "wo": wo, "w1": w1, "w2": w2,
            "mask4": mask4, "ident": ident, "ones": ones,
        })
    return in_maps


def kernel(x, w_qkv, w_out, mask, _trace=False):
    """Full MHA forward. Returns [1, L, D] float32."""
    nc = _get_nc()
    in_maps = prep_inputs(x, w_qkv, w_out)
    res = run_bass_kernel_spmd(nc, in_maps, core_ids=list(range(N_CORES)),
                               trace=_trace)
    acc = np.zeros((L, D), dtype=np.float32)
    for r in res.results:
        acc += r["out"]
    out = acc.reshape(1, L, D)
    if _trace:
        return out, res
    return out
